# revision 1
# baseline (speedup 1.0000x reference)
"""BatchHardTripletLoss on 8 Trainium2 NeuronCores — fp8 DoubleRow version.

Strategy (row-parallel per the sharding hint), heavy prep on host:
  - Host: sort rows by label (loss is a mean over anchors, permutation
    invariant); L2-normalize in f32; cast to fp8-e4m3 (measured 7.3e-4
    rel err on the final loss vs the 2e-2 gate); build per-core
    transposed operand layouts [128, 2, cols] with k = 128*j + p so the
    PE's fp8 DoubleRow mode contracts the full K=256 in one pass at
    0.5 cycles/output-column (2x the fp32r rate).
  - Candidates are rolled per core so the core's 1024 anchors sit at
    local columns [256, 1280): every anchor tile m's positive range
    then lands inside a fixed 256-wide window [128m+192, 128m+448)
    (max class size 29 << 64 margin; asserted on host).
  - Positives are excluded from the hardest-negative max by a penalty
    matmul: diag(-128) fp8 stationary times a 0/1 mask accumulates
    -128 onto exactly the positive columns of the bulk PSUM gram.  The
    SAME penalized PSUM also yields hardest_pos: the min over the
    window columns is attained on the penalized range, and +128
    recovers min_pos(g) exactly (PSUM is f32).  No separate window
    gram, no Phase A normalization/transposes on device at all.
  - Per [128, 1024] PSUM group (two 512-col DoubleRow matmuls), the
    drain is split across the only two engines that can read PSUM on
    TRN2 (GPSIMD has no PSUM port; DMA is SBUF<->DRAM only): ~41
    groups are evicted by Act to fp16 SBUF where DVE max-accumulates
    at the 4x packed-16-bit rate (~0.26 ns/elem); ~23 groups are
    reduced by DVE directly from PSUM (tensor_reduce, 1x).  One merged
    [128, 1] partial per group — per-512-tile partials are pointless.
    Window groups also get a DVE min over the penalized window columns
    read from f32 PSUM (the fp16 eviction would quantize the
    -128-offset values to ~0.06, so the min must see PSUM).
  - Epilogue: d = sqrt(clip(2 - 2g, eps)) with one Newton step, loss =
    relu(hp - hn + 0.3), row-sum, cross-partition sum via ones-matmul;
    host averages the 8 per-core sums.  A dummy sqrt at build start
    pins the sqrt_and_others act table (which also serves Copy) so the
    epilogue pays no table switch.
"""

import numpy as np
import ml_dtypes

N = 8192
D = 256
NCORES = 8
CA = N // NCORES          # anchors per core
MT = CA // 128            # 8 anchor tiles per core
NT = N // 512             # 16 column tiles of 512
NH = N // 1024            # 8 psum groups of 1024
WOFF = 192                # window start offset: local cols [128m+WOFF, +WWID)
WWID = 256
BIG = 128.0               # fp8-exact penalty magnitude
GUARD = 256               # anchors sit at local cols [GUARD, GUARD+CA)

F8NP = ml_dtypes.float8_e4m3

_CACHE = {}


def _window(m):
    w0 = 128 * m + WOFF
    return w0, w0 + WWID


def _roles():
    """(schedule, roles) over 1024-col groups (h, m) — h covers local
    cols [1024h, 1024h+1024) as a [128, 2, 512] PSUM tile (2 banks,
    4 in flight).  Each group gets ONE merged consumer op producing a
    single [128, 1] partial (per-tile partials are unnecessary):
      'W': Act-evict fp16 + DVE max-accum + DVE window min from PSUM
      'A': Act-evict fp16 + cleanup max-accum (DVE fast path, Pool for
           every 3rd — GPSIMD cannot touch PSUM on TRN2 and DMA is
           SBUF<->DRAM only, so PSUM drains ONLY through Act + DVE;
           Pool helps post-SBUF)
      'D': DVE tensor_reduce (axis XY) direct from PSUM
    Window groups (windows lie in h=0,1) are interleaved among bulk."""
    roles = {}
    wq, nonw = [], []
    for h in range(NH):
        for m in range(MT):
            w0, w1 = _window(m)
            if w0 < 1024 * (h + 1) and w1 > 1024 * h:
                roles[(h, m)] = "W"
                wq.append((h, m))
            else:
                nonw.append((h, m))
    # LP-balanced targets: ~41 evictions (incl windows), ~23
    # DVE-direct.  (Pool/GPSIMD is out entirely: no PSUM access, and
    # walrus rejects tensor_scalar+accum on Pool.)
    import os
    na = int(os.environ.get("BHT_A", "41"))
    want = {"A": na - len(wq), "D": 64 - na}
    emitted = {"A": 0, "D": 0}
    for gm in nonw:
        r = max(want, key=lambda k: want[k] / (emitted[k] + 1))
        roles[gm] = r
        emitted[r] += 1
    schedule = []
    wi = 0
    for i, gm in enumerate(nonw):
        schedule.append(gm)
        if i % 5 == 4 and wi < len(wq):
            schedule.append(wq[wi])
            wi += 1
    schedule.extend(wq[wi:])
    return schedule, roles


def _build(reps=1):
    import concourse.bass as bass  # noqa: F401
    import concourse.tile as tile
    from concourse import bacc, mybir, masks
    from contextlib import ExitStack

    F32 = mybir.dt.float32
    F16 = mybir.dt.float16
    F8 = mybir.dt.float8e4
    OP = mybir.AluOpType
    AX = mybir.AxisListType
    PM = mybir.MatmulPerfMode.DoubleRow

    nc = bacc.Bacc("TRN2", target_bir_lowering=False, debug=False,
                   num_devices=NCORES)

    ft_d = nc.dram_tensor("ft", [128, 2, N], F8, kind="ExternalInput").ap()
    at_d = nc.dram_tensor("at", [128, 2, CA], F8, kind="ExternalInput").ap()
    wm_d = nc.dram_tensor("wm", [128, MT, WWID], F8,
                          kind="ExternalInput").ap()
    out_d = nc.dram_tensor("out", [1, 1], F32, kind="ExternalOutput").ap()

    schedule, roles = _roles()

    with tile.TileContext(nc) as tc:
      for _rep in range(reps):
        with ExitStack() as ctx:
            constp = ctx.enter_context(tc.tile_pool(name="const", bufs=1))
            ident = constp.tile([128, 128], F32)
            masks.make_identity(nc, ident[:])
            ones = constp.tile([128, 1], F32)
            nc.gpsimd.memset(ones[:], 1.0)
            negid = constp.tile([128, 128], F8)
            nc.scalar.mul(negid[:], ident[:], -BIG)
            # Dummy sqrt: steer the act-table pass to load sqrt_and_others
            # (which also contains Copy) once, up front — no reload for
            # the epilogue sqrt.
            warm = constp.tile([128, 1], F32)
            nc.scalar.sqrt(warm[:], ones[:])

            vecp = ctx.enter_context(tc.tile_pool(name="vec", bufs=1))
            bp2 = vecp.tile([128, MT, NH], F32)   # per-group row maxima
            wmn = vecp.tile([128, MT, 2], F32)    # window min partials
            nc.gpsimd.memset(wmn[:], 1000.0)

            bigp = ctx.enter_context(tc.tile_pool(name="big", bufs=1))
            ft = bigp.tile([128, 2, N], F8)
            at = bigp.tile([128, 2, CA], F8)
            wm = bigp.tile([128, MT, WWID], F8)

            # at + first ft chunk first: the first scheduled groups only
            # need those; wm is only needed once window groups start.
            nc.sync.dma_start(at[:], at_d[:])
            nc.sync.dma_start(ft[:, :, 0:1024], ft_d[:, :, 0:1024])
            nc.sync.dma_start(wm[:], wm_d[:])
            nc.sync.dma_start(ft[:, :, 1024:2048], ft_d[:, :, 1024:2048])
            for q in range(1, 4):
                nc.sync.dma_start(ft[:, :, 2048 * q:2048 * (q + 1)],
                                  ft_d[:, :, 2048 * q:2048 * (q + 1)])

            with ExitStack() as bctx:
                psB = bctx.enter_context(
                    tc.tile_pool(name="psB", bufs=4, space="PSUM"))
                evp = bctx.enter_context(tc.tile_pool(name="ev", bufs=8))
                scrp = bctx.enter_context(tc.tile_pool(name="scr", bufs=4))

                wj = [0] * MT  # next window-partial slot per m

                def group(h, m):
                    role = roles[(h, m)]
                    lh = at[:, :, 128 * m:128 * (m + 1)]
                    w0, w1 = _window(m)
                    ps = psB.tile([128, 2, 512], F32, tag="g")
                    for j in range(2):
                        t = 2 * h + j
                        c0, c1 = 512 * t, 512 * (t + 1)
                        lo, hi = max(w0, c0), min(w1, c1)
                        nc.tensor.matmul(
                            ps[:, j, :], lh, ft[:, :, c0:c1],
                            start=True, stop=(lo >= hi), perf_mode=PM)
                        if lo < hi:
                            nc.tensor.matmul(
                                ps[:, j, lo - c0:hi - c0],
                                negid[:], wm[:, m, lo - w0:hi - w0],
                                start=False, stop=True)
                    dst = bp2[:, m, h:h + 1]
                    if role in ("A", "W"):
                        # fp16 eviction is safe for the max: penalized
                        # columns stay ~-127 and never win the max.
                        # (tensor_scalar+accum gets the fast DVE mode on
                        # packed fp16 SBUF; tensor_reduce does not.)
                        ev = evp.tile([128, 2, 512], F16, tag="ev")
                        nc.scalar.copy(ev[:], ps[:])
                        evf = ev[:].rearrange("p a b -> p (a b)")
                        nc.vector.tensor_scalar(
                            evf, evf, 0.0, None, OP.add, OP.max,
                            accum_out=dst)
                    else:  # 'D'
                        nc.vector.tensor_reduce(
                            dst, ps[:], axis=AX.XY, op=OP.max)
                    if role == "W":
                        # hardest-pos: min over the penalized window read
                        # from f32 PSUM (fp16 would quantize -128-offset
                        # values to ~0.06).  Window may straddle 2 tiles.
                        lo0, hi0 = max(w0, 1024 * h), min(w1, 1024 * (h + 1))
                        for t in range(lo0 // 512, (hi0 - 1) // 512 + 1):
                            c0 = 512 * t
                            lo, hi = max(lo0, c0), min(hi0, c0 + 512)
                            scr = scrp.tile([128, WWID], F32, tag="scr")
                            j = wj[m]
                            wj[m] += 1
                            nc.vector.tensor_scalar(
                                scr[:, :hi - lo],
                                ps[:, t - 2 * h, lo - c0:hi - c0],
                                0.0, None, OP.add, OP.min,
                                accum_out=wmn[:, m, j:j + 1])

                for h, m in schedule:
                    group(h, m)

            # ---------------- epilogue ----------------------------------
            with ExitStack() as cctx:
                ep = cctx.enter_context(tc.tile_pool(name="ep", bufs=1))
                psC = cctx.enter_context(
                    tc.tile_pool(name="psC", bufs=1, space="PSUM"))

                negg = ep.tile([128, MT], F32)
                nc.vector.tensor_reduce(
                    negg[:], bp2[:], axis=AX.X, op=OP.max)
                wpos = ep.tile([128, MT], F32)
                nc.vector.tensor_reduce(
                    wpos[:], wmn[:], axis=AX.X, op=OP.min)

                # hp2 = clip(2 - 2*(wpos+BIG)); hn2 = clip(2 - 2*negg)
                hh2 = ep.tile([128, 2, MT], F32)
                nc.vector.tensor_scalar(
                    hh2[:, 0, :], wpos[:], -2.0, 2.0 - 2.0 * BIG,
                    OP.mult, OP.add)
                nc.vector.tensor_scalar(
                    hh2[:, 1, :], negg[:], -2.0, 2.0, OP.mult, OP.add)
                nc.vector.tensor_scalar_max(hh2[:, :, :], hh2[:, :, :],
                                            1e-12)

                hhf = hh2[:].rearrange("p a b -> p (a b)")
                y = ep.tile([128, 2 * MT], F32)
                nc.scalar.sqrt(y[:], hhf)
                # one Newton step: y' = 0.5*(y + x/y)
                ry = ep.tile([128, 2 * MT], F32)
                nc.vector.reciprocal(ry[:], y[:])
                nc.vector.tensor_mul(ry[:], ry[:], hhf)
                nc.vector.tensor_add(ry[:], ry[:], y[:])
                nc.vector.tensor_scalar_mul(ry[:], ry[:], 0.5)

                ryv = ry[:].rearrange("p (a b) -> p a b", a=2)
                loss = ep.tile([128, MT], F32)
                nc.vector.tensor_sub(loss[:], ryv[:, 0, :], ryv[:, 1, :])
                nc.vector.tensor_scalar(
                    loss[:], loss[:], 0.3, 0.0, OP.add, OP.max)

                rowsum = ep.tile([128, 1], F32)
                nc.vector.tensor_reduce(
                    rowsum[:], loss[:], axis=AX.X, op=OP.add)
                tot = psC.tile([1, 1], F32)
                nc.tensor.matmul(tot[:], rowsum[:], ones[:],
                                 start=True, stop=True)
                osb = ep.tile([1, 1], F32)
                nc.scalar.copy(osb[:], tot[:])
                nc.sync.dma_start(out_d[:], osb[:])

    nc.compile()
    return nc


def _prep_inputs(features, labels):
    feats = np.asarray(features, dtype=np.float32)
    labs = np.asarray(labels)
    order = np.argsort(labs, kind="stable")
    sf = feats[order]
    sl = labs[order]
    nrm = np.maximum(np.sqrt((sf * sf).sum(1, keepdims=True)), 1e-12)
    f8 = (sf / nrm).astype(F8NP)
    s_g = np.searchsorted(sl, sl, side="left")
    e_g = np.searchsorted(sl, sl, side="right")

    # [128, 2, N] with value[p, j, col] = f8[col, 128j + p]
    tmp = np.ascontiguousarray(f8.reshape(N, 2, 128).transpose(2, 1, 0))
    jj = np.arange(WWID)
    in_maps = []
    for c in range(NCORES):
        b = 1024 * c - GUARD
        ft_c = np.roll(tmp, -b, axis=2)
        at_c = np.ascontiguousarray(tmp[:, :, 1024 * c:1024 * (c + 1)])
        wm_c = np.zeros((128, MT, WWID), F8NP)
        for m in range(MT):
            i0 = 1024 * c + 128 * m
            ls = s_g[i0:i0 + 128] - b
            le = e_g[i0:i0 + 128] - b
            w0, w1 = _window(m)
            assert (ls >= w0).all() and (le <= w1).all() and (ls < le).all(), \
                f"window containment violated c={c} m={m}"
            wm_c[:, m, :] = (
                (jj[None, :] >= (ls - w0)[:, None])
                & (jj[None, :] < (le - w0)[:, None])).astype(F8NP)
        in_maps.append({"ft": ft_c, "at": at_c, "wm": wm_c})
    return in_maps


def kernel(features, labels):
    from concourse.bass_utils import run_bass_kernel_spmd

    if "nc" not in _CACHE:
        _CACHE["nc"] = _build()
    nc = _CACHE["nc"]

    in_maps = _prep_inputs(features, labels)
    res = run_bass_kernel_spmd(nc, in_maps, core_ids=list(range(NCORES)))
    total = np.float64(0.0)
    for c in range(NCORES):
        total += np.float64(res.results[c]["out"].reshape(())[()])
    return np.float32(total / N)



# revision 7
# speedup vs baseline: 1.5829x; 1.5829x over previous
"""BatchHardTripletLoss on 8 Trainium2 NeuronCores — fp8 DoubleRow + LSE drain.

Strategy (row-parallel per the sharding hint), heavy prep on host:
  - Host: sort rows by label (loss is a mean over anchors, permutation
    invariant); L2-normalize in f32; cast to fp8-e4m3; build per-core
    transposed operand layouts [128, 2, cols] with k = 128*j + p so the
    PE's fp8 DoubleRow mode contracts the full K=256 in one pass at
    0.5 cycles/output-column.
  - Candidates are rolled per core so the core's 1024 anchors sit at
    local columns [256, 1280): every anchor tile m's positive range
    lands inside a fixed 256-wide window [128m+192, 128m+448).
  - Positives are excluded from the hardest-negative reduction by a
    penalty matmul on window groups only: diag(-128) fp8 stationary
    times a 0/1 mask accumulates -128 onto exactly the positive columns
    of the PSUM gram.
  - PSUM drain (the bottleneck: every gram element must leave PSUM
    through Act at 0.83 ns/col or DVE at 1.04 ns/col; DMA and GPSIMD
    have no PSUM path, walrus only allows ONE PSUM operand per DVE op):
      'E' groups: ONE Act instruction — in-place exp(BETA*(g-CEN)) on
          the PSUM tile with accum_out summing the 1024 columns
          (~1.18 us, no DVE work).  log-sum-exp over-approximates the
          row max by <2.1e-3 at BETA=500 (measured), and the -128
          penalty makes positive columns vanish (exp -> 0).
      'D' groups: ONE DVE tensor_reduce max direct from PSUM (~1.19 us)
          — exact.
      'W' groups (positive window overlaps): DVE max + DVE min over the
          penalized window columns (hardest-pos via min_pos(g)+128).
    Both engines run ~full tilt on disjoint groups; 4 [128,2,512] PSUM
    tiles (8 banks) keep the pipeline fed.
  - Epilogue merges: hn_g = max(CEN + ln(sum S_h + sum exp(BETA*(M_h -
    CEN)))/BETA applied via affine ops); hp from the window minima;
    d = sqrt(clip(2-2g)) computed as exp(0.5*ln(x)) + one Newton step so
    the WHOLE kernel uses the single natural_log_exp_and_others act
    table (a Bacc subclass pins it; the stock greedy pass would thrash
    exp_and_others/natural_log loads at 1.28 us each).
  - loss = relu(hp - hn + 0.3), row-sum, cross-partition sum via
    ones-matmul; host averages the 8 per-core sums.
"""

import os

import numpy as np
import ml_dtypes

N = 8192
D = 256
NCORES = 8
CA = N // NCORES          # anchors per core
MT = CA // 128            # 8 anchor tiles per core
NH = N // 1024            # 8 psum groups of 1024 per anchor tile
WOFF = 192                # window start offset: local cols [128m+WOFF, +WWID)
WWID = 256
BIG = 128.0               # fp8-exact penalty magnitude
GUARD = 256               # anchors sit at local cols [GUARD, GUARD+CA)

BETA = 500.0              # LSE sharpness (bias < 2.1e-3 on g, measured)
CEN = 0.28                # LSE center: overflow-safe for g_neg <= CEN+88/BETA

F8NP = ml_dtypes.float8_e4m3

_CACHE = {}


def _window(m):
    w0 = 128 * m + WOFF
    return w0, w0 + WWID


def _roles():
    """(schedule, roles) over 1024-col groups (h, m).
      'W': DVE max + DVE window min from f32 PSUM (penalized groups)
      'E': Act in-place exp+accum (LSE partial), no DVE work
      'D': DVE tensor_reduce max direct from PSUM (exact)
    E/D interleaved so Act and DVE drain different groups concurrently."""
    roles = {}
    wq, nonw = [], []
    for h in range(NH):
        for m in range(MT):
            w0, w1 = _window(m)
            if w0 < 1024 * (h + 1) and w1 > 1024 * h:
                roles[(h, m)] = "W"
                wq.append((h, m))
            else:
                nonw.append((h, m))
    ne = int(os.environ.get("BHT_E", "34"))
    want = {"D": len(nonw) - ne, "E": ne}
    emitted = {"D": 0, "E": 0}
    for gm in nonw:
        # iteration order of `want` breaks ties toward D, so the schedule
        # opens with a D group and DVE starts draining immediately.
        r = max(want, key=lambda k: want[k] / (emitted[k] + 1))
        roles[gm] = r
        emitted[r] += 1
    schedule = []
    wi = 0
    for i, gm in enumerate(nonw):
        schedule.append(gm)
        if i % 5 == 4 and wi < len(wq):
            schedule.append(wq[wi])
            wi += 1
    schedule.extend(wq[wi:])
    return schedule, roles


def _build(reps=1):
    import concourse.bass as bass  # noqa: F401
    import concourse.tile as tile
    from concourse import bacc, mybir, masks
    from contextlib import ExitStack

    F32 = mybir.dt.float32
    F8 = mybir.dt.float8e4
    OP = mybir.AluOpType
    AX = mybir.AxisListType
    ACT = mybir.ActivationFunctionType
    PM = mybir.MatmulPerfMode.DoubleRow

    class PinnedBacc(bacc.Bacc):
        """Force the act-table pass onto the one table containing both
        exp and ln (plus copy/identity); the stock greedy first-match
        would alternate exp_and_others / natural_log loads."""

        def insert_act_table_loads(self):
            from concourse.hw_specs import get_activation_tables
            import bass_rust as _bass_rust

            has_act = any(
                isinstance(i, mybir.InstActivation)
                for b in self.main_func.blocks
                for i in b.instructions
            )
            if not has_act:
                return
            tables = list(get_activation_tables(self.m.arch).items())
            tables = [
                (n, (f if n == "natural_log_exp_and_others" else set()))
                for n, f in tables
            ]
            _bass_rust.insert_act_table_loads(self, tables)

    nc = PinnedBacc("TRN2", target_bir_lowering=False, debug=False,
                    num_devices=NCORES)

    ft_d = nc.dram_tensor("ft", [128, 2, N], F8, kind="ExternalInput").ap()
    wm_d = nc.dram_tensor("wm", [128, MT, WWID], F8,
                          kind="ExternalInput").ap()
    out_d = nc.dram_tensor("out", [1, 1], F32, kind="ExternalOutput").ap()

    schedule, roles = _roles()

    with tile.TileContext(nc) as tc:
      for _rep in range(reps):
        with ExitStack() as ctx:
            constp = ctx.enter_context(tc.tile_pool(name="const", bufs=1))
            ident = constp.tile([128, 128], F32)
            masks.make_identity(nc, ident[:])
            ones = constp.tile([128, 1], F32)
            nc.gpsimd.memset(ones[:], 1.0)
            negid = constp.tile([128, 128], F8)
            nc.scalar.mul(negid[:], ident[:], -BIG)
            ebias = constp.tile([128, 1], F32)
            nc.gpsimd.memset(ebias[:], -BETA * CEN)
            zbias = constp.tile([128, 1], F32)
            nc.gpsimd.memset(zbias[:], 0.0)

            vecp = ctx.enter_context(tc.tile_pool(name="vec", bufs=1))
            bp2 = vecp.tile([128, MT, NH], F32)   # exact per-group maxima
            sp2 = vecp.tile([128, MT, NH], F32)   # LSE per-group sums
            wmn = vecp.tile([128, MT, 2], F32)    # window min partials
            nc.gpsimd.memset(bp2[:], -1000.0)
            nc.gpsimd.memset(sp2[:], 0.0)
            nc.gpsimd.memset(wmn[:], 1000.0)

            bigp = ctx.enter_context(tc.tile_pool(name="big", bufs=1))
            ft = bigp.tile([128, 2, N], F8)
            wm = bigp.tile([128, MT, WWID], F8)

            # Anchors live at cols [GUARD, GUARD+CA) = [256, 1280): the
            # first chunk covers them plus all of h=0's candidates.
            nc.sync.dma_start(ft[:, :, 0:1280], ft_d[:, :, 0:1280])
            nc.sync.dma_start(wm[:], wm_d[:])
            nc.sync.dma_start(ft[:, :, 1280:3072], ft_d[:, :, 1280:3072])
            nc.sync.dma_start(ft[:, :, 3072:4864], ft_d[:, :, 3072:4864])
            nc.sync.dma_start(ft[:, :, 4864:6656], ft_d[:, :, 4864:6656])
            nc.sync.dma_start(ft[:, :, 6656:8192], ft_d[:, :, 6656:8192])

            with ExitStack() as bctx:
                psB = bctx.enter_context(
                    tc.tile_pool(name="psB", bufs=4, space="PSUM"))
                scrp = bctx.enter_context(tc.tile_pool(name="scr", bufs=4))

                wj = [0] * MT  # next window-partial slot per m

                def group(h, m):
                    role = roles[(h, m)]
                    a0 = GUARD + 128 * m
                    lh = ft[:, :, a0:a0 + 128]
                    w0, w1 = _window(m)
                    ps = psB.tile([128, 2, 512], F32, tag="g")
                    for j in range(2):
                        t = 2 * h + j
                        c0, c1 = 512 * t, 512 * (t + 1)
                        lo, hi = max(w0, c0), min(w1, c1)
                        pen = role == "W" and lo < hi
                        nc.tensor.matmul(
                            ps[:, j, :], lh, ft[:, :, c0:c1],
                            start=True, stop=not pen, perf_mode=PM)
                        if pen:
                            nc.tensor.matmul(
                                ps[:, j, lo - c0:hi - c0],
                                negid[:], wm[:, m, lo - w0:hi - w0],
                                start=False, stop=True)
                    if role == "E":
                        nc.scalar.activation(
                            ps[:], ps[:], ACT.Exp, bias=ebias[:],
                            scale=BETA, accum_out=sp2[:, m, h:h + 1])
                        return
                    nc.vector.tensor_reduce(
                        bp2[:, m, h:h + 1], ps[:], axis=AX.XY, op=OP.max)
                    if role == "W":
                        # hardest-pos: min over the penalized window from
                        # f32 PSUM.  The window is contiguous in the
                        # flattened [128, 1024] view even when it straddles
                        # the two 512-col halves.
                        lo0, hi0 = max(w0, 1024 * h), min(w1, 1024 * (h + 1))
                        psf = ps[:].rearrange("p a b -> p (a b)")
                        scr = scrp.tile([128, WWID], F32, tag="scr")
                        j = wj[m]
                        wj[m] += 1
                        nc.vector.tensor_scalar(
                            scr[:, :hi0 - lo0],
                            psf[:, lo0 - 1024 * h:hi0 - 1024 * h],
                            0.0, None, OP.add, OP.min,
                            accum_out=wmn[:, m, j:j + 1])

                for h, m in schedule:
                    group(h, m)

            # ---------------- epilogue ----------------------------------
            with ExitStack() as cctx:
                ep = cctx.enter_context(tc.tile_pool(name="ep", bufs=1))
                psC = cctx.enter_context(
                    tc.tile_pool(name="psC", bufs=1, space="PSUM"))

                negS = ep.tile([128, MT], F32)
                nc.vector.tensor_reduce(
                    negS[:], sp2[:], axis=AX.X, op=OP.add)
                negM = ep.tile([128, MT], F32)
                nc.vector.tensor_reduce(
                    negM[:], bp2[:], axis=AX.X, op=OP.max)
                expM = ep.tile([128, MT], F32)
                nc.scalar.activation(expM[:], negM[:], ACT.Exp,
                                     bias=ebias[:], scale=BETA)
                nc.vector.tensor_add(negS[:], negS[:], expM[:])
                lnS = ep.tile([128, MT], F32)
                nc.scalar.activation(lnS[:], negS[:], ACT.Ln,
                                     bias=zbias[:], scale=1.0)

                wpos = ep.tile([128, MT], F32)
                nc.vector.tensor_reduce(
                    wpos[:], wmn[:], axis=AX.X, op=OP.min)

                # hp2 = clip(2 - 2*(wpos+BIG)); hn2 = clip(2-2*CEN - (2/B)lnS)
                hh2 = ep.tile([128, 2, MT], F32)
                nc.vector.tensor_scalar(
                    hh2[:, 0, :], wpos[:], -2.0, 2.0 - 2.0 * BIG,
                    OP.mult, OP.add)
                nc.vector.tensor_scalar(
                    hh2[:, 1, :], lnS[:], -2.0 / BETA, 2.0 - 2.0 * CEN,
                    OP.mult, OP.add)
                nc.vector.tensor_scalar_max(hh2[:, :, :], hh2[:, :, :],
                                            1e-12)

                # sqrt(x) = exp(0.5 ln x) — measured 1.2e-7 rel on HW,
                # no Newton step needed
                hhf = hh2[:].rearrange("p a b -> p (a b)")
                lnh = ep.tile([128, 2 * MT], F32)
                nc.scalar.activation(lnh[:], hhf, ACT.Ln,
                                     bias=zbias[:], scale=1.0)
                ry = ep.tile([128, 2 * MT], F32)
                nc.scalar.activation(ry[:], lnh[:], ACT.Exp,
                                     bias=zbias[:], scale=0.5)

                ryv = ry[:].rearrange("p (a b) -> p a b", a=2)
                loss = ep.tile([128, MT], F32)
                nc.vector.tensor_sub(loss[:], ryv[:, 0, :], ryv[:, 1, :])
                nc.vector.tensor_scalar(
                    loss[:], loss[:], 0.3, 0.0, OP.add, OP.max)

                rowsum = ep.tile([128, 1], F32)
                nc.vector.tensor_reduce(
                    rowsum[:], loss[:], axis=AX.X, op=OP.add)
                tot = psC.tile([1, 1], F32)
                nc.tensor.matmul(tot[:], rowsum[:], ones[:],
                                 start=True, stop=True)
                osb = ep.tile([1, 1], F32)
                nc.scalar.copy(osb[:], tot[:])
                nc.sync.dma_start(out_d[:], osb[:])

    nc.compile()
    return nc


def _prep_inputs(features, labels):
    feats = np.asarray(features, dtype=np.float32)
    labs = np.asarray(labels)
    order = np.argsort(labs, kind="stable")
    sf = feats[order]
    sl = labs[order]
    nrm = np.maximum(np.sqrt((sf * sf).sum(1, keepdims=True)), 1e-12)
    f8 = (sf / nrm).astype(F8NP)
    s_g = np.searchsorted(sl, sl, side="left")
    e_g = np.searchsorted(sl, sl, side="right")

    # [128, 2, N] with value[p, j, col] = f8[col, 128j + p]
    tmp = np.ascontiguousarray(f8.reshape(N, 2, 128).transpose(2, 1, 0))
    jj = np.arange(WWID)
    in_maps = []
    for c in range(NCORES):
        b = 1024 * c - GUARD
        ft_c = np.roll(tmp, -b, axis=2)
        wm_c = np.zeros((128, MT, WWID), F8NP)
        for m in range(MT):
            i0 = 1024 * c + 128 * m
            ls = s_g[i0:i0 + 128] - b
            le = e_g[i0:i0 + 128] - b
            w0, w1 = _window(m)
            assert (ls >= w0).all() and (le <= w1).all() and (ls < le).all(), \
                f"window containment violated c={c} m={m}"
            wm_c[:, m, :] = (
                (jj[None, :] >= (ls - w0)[:, None])
                & (jj[None, :] < (le - w0)[:, None])).astype(F8NP)
        in_maps.append({"ft": ft_c, "wm": wm_c})
    return in_maps


def kernel(features, labels):
    from concourse.bass_utils import run_bass_kernel_spmd

    if "nc" not in _CACHE:
        _CACHE["nc"] = _build()
    nc = _CACHE["nc"]

    in_maps = _prep_inputs(features, labels)
    res = run_bass_kernel_spmd(nc, in_maps, core_ids=list(range(NCORES)))
    total = np.float64(0.0)
    for c in range(NCORES):
        total += np.float64(res.results[c]["out"].reshape(())[()])
    return np.float32(total / N)


# revision 9
# speedup vs baseline: 1.8300x; 1.1561x over previous
"""BatchHardTripletLoss on 8 Trainium2 NeuronCores — fp8 DoubleRow + LSE drain.

Strategy (row-parallel per the sharding hint), heavy prep on host:
  - Host: sort rows by label (loss is a mean over anchors, permutation
    invariant); L2-normalize in f32; cast to fp8-e4m3; build per-core
    transposed operand layouts [128, 2, cols] with k = 128*j + p so the
    PE's fp8 DoubleRow mode contracts the full K=256 in one pass at
    0.5 cycles/output-column.
  - Candidates are rolled per core so the core's 1024 anchors sit at
    local columns [256, 1280): every anchor tile m's positive range
    lands inside a fixed 256-wide window [128m+192, 128m+448).
  - Positives are excluded from the hardest-negative reduction by a
    penalty matmul on window groups only: diag(-128) fp8 stationary
    times a 0/1 mask accumulates -128 onto exactly the positive columns
    of the PSUM gram.
  - PSUM drain (the bottleneck: every gram element must leave PSUM
    through Act at 0.83 ns/col or DVE at 1.04 ns/col; DMA and GPSIMD
    have no PSUM path, walrus only allows ONE PSUM operand per DVE op):
      'E' groups: ONE Act instruction — in-place exp(BETA*(g-CEN)) on
          the PSUM tile with accum_out summing the 1024 columns
          (~1.18 us, no DVE work).  log-sum-exp over-approximates the
          row max by <2.1e-3 at BETA=500 (measured), and the -128
          penalty makes positive columns vanish (exp -> 0).
      'D' groups: ONE DVE tensor_reduce max direct from PSUM (~1.19 us)
          — exact.
      'W' groups (positive window overlaps): DVE max + DVE min over the
          penalized window columns (hardest-pos via min_pos(g)+128).
    Both engines run ~full tilt on disjoint groups; 4 [128,2,512] PSUM
    tiles (8 banks) keep the pipeline fed.
  - Epilogue merges: hn_g = max(CEN + ln(sum S_h + sum exp(BETA*(M_h -
    CEN)))/BETA applied via affine ops); hp from the window minima;
    d = sqrt(clip(2-2g)) computed as exp(0.5*ln(x)) + one Newton step so
    the WHOLE kernel uses the single natural_log_exp_and_others act
    table (a Bacc subclass pins it; the stock greedy pass would thrash
    exp_and_others/natural_log loads at 1.28 us each).
  - loss = relu(hp - hn + 0.3), row-sum, cross-partition sum via
    ones-matmul; host averages the 8 per-core sums.
"""

import os

import numpy as np
import ml_dtypes

N = 8192
D = 256
NCORES = 8
CA = N // NCORES          # anchors per core
MT = CA // 128            # 8 anchor tiles per core
NH = N // 1024            # 8 psum groups of 1024 per anchor tile
WOFF = 192                # window start offset: local cols [128m+WOFF, +WWID)
WWID = 256
BIG = 128.0               # fp8-exact penalty magnitude
GUARD = 256               # anchors sit at local cols [GUARD, GUARD+CA)

BETA = 300.0              # LSE sharpness (bias ~3.5e-3 on g)
CEN = 0.35                # centers log2(S) in the Ln table's valid window:
                          # measured on HW, Ln is exact only for |log2 x| <= 64
                          # (garbage above 2^66!); S spans 2^-56..2^56 here

F8NP = ml_dtypes.float8_e4m3

_CACHE = {}


def _window(m):
    w0 = 128 * m + WOFF
    return w0, w0 + WWID


def _roles():
    """(schedule, roles) over 1024-col groups (h, m).
      'W': DVE max + DVE window min from f32 PSUM (penalized groups)
      'E': Act in-place exp+accum (LSE partial), no DVE work
      'D': DVE tensor_reduce max direct from PSUM (exact)
    E/D interleaved so Act and DVE drain different groups concurrently."""
    roles = {}
    wq, nonw = [], []
    for h in range(NH):
        for m in range(MT):
            w0, w1 = _window(m)
            if w0 < 1024 * (h + 1) and w1 > 1024 * h:
                roles[(h, m)] = "W"
                wq.append((h, m))
            else:
                nonw.append((h, m))
    ne = int(os.environ.get("BHT_E", "34"))
    want = {"D": len(nonw) - ne, "E": ne}
    emitted = {"D": 0, "E": 0}
    for gm in nonw:
        # iteration order of `want` breaks ties toward D, so the schedule
        # opens with a D group and DVE starts draining immediately.
        r = max(want, key=lambda k: want[k] / (emitted[k] + 1))
        roles[gm] = r
        emitted[r] += 1
    schedule = []
    wi = 0
    for i, gm in enumerate(nonw):
        schedule.append(gm)
        if i % 5 == 4 and wi < len(wq):
            schedule.append(wq[wi])
            wi += 1
    schedule.extend(wq[wi:])
    return schedule, roles


def _build(reps=1):
    import concourse.bass as bass  # noqa: F401
    import concourse.tile as tile
    from concourse import bacc, mybir, masks
    from contextlib import ExitStack

    F32 = mybir.dt.float32
    F8 = mybir.dt.float8e4
    OP = mybir.AluOpType
    AX = mybir.AxisListType
    ACT = mybir.ActivationFunctionType
    PM = mybir.MatmulPerfMode.DoubleRow

    class PinnedBacc(bacc.Bacc):
        """Force the act-table pass onto the one table containing both
        exp and ln (plus copy/identity); the stock greedy first-match
        would alternate exp_and_others / natural_log loads."""

        def insert_act_table_loads(self):
            from concourse.hw_specs import get_activation_tables
            import bass_rust as _bass_rust

            has_act = any(
                isinstance(i, mybir.InstActivation)
                for b in self.main_func.blocks
                for i in b.instructions
            )
            if not has_act:
                return
            tables = list(get_activation_tables(self.m.arch).items())
            tables = [
                (n, (f if n == "natural_log_exp_and_others" else set()))
                for n, f in tables
            ]
            _bass_rust.insert_act_table_loads(self, tables)

    nc = PinnedBacc("TRN2", target_bir_lowering=False, debug=False,
                    num_devices=NCORES)

    ft_d = nc.dram_tensor("ft", [128, 2, N], F8, kind="ExternalInput").ap()
    wm_d = nc.dram_tensor("wm", [128, MT, WWID], F8,
                          kind="ExternalInput").ap()
    out_d = nc.dram_tensor("out", [1, 1], F32, kind="ExternalOutput").ap()

    schedule, roles = _roles()

    with tile.TileContext(nc) as tc:
      for _rep in range(reps):
        with ExitStack() as ctx:
            constp = ctx.enter_context(tc.tile_pool(name="const", bufs=1))
            ident = constp.tile([128, 128], F32)
            masks.make_identity(nc, ident[:])
            ones = constp.tile([128, 1], F32)
            nc.gpsimd.memset(ones[:], 1.0)
            negid = constp.tile([128, 128], F8)
            nc.scalar.mul(negid[:], ident[:], -BIG)
            ebias = constp.tile([128, 1], F32)
            nc.gpsimd.memset(ebias[:], -BETA * CEN)
            zbias = constp.tile([128, 1], F32)
            nc.gpsimd.memset(zbias[:], 0.0)

            vecp = ctx.enter_context(tc.tile_pool(name="vec", bufs=1))
            bp2 = vecp.tile([128, MT, NH], F32)   # exact per-group maxima
            sp2 = vecp.tile([128, MT, NH], F32)   # LSE per-group sums
            wmn = vecp.tile([128, MT, 2], F32)    # window min partials
            nc.gpsimd.memset(bp2[:], -1000.0)
            nc.gpsimd.memset(sp2[:], 0.0)
            nc.gpsimd.memset(wmn[:], 1000.0)

            bigp = ctx.enter_context(tc.tile_pool(name="big", bufs=1))
            ft = bigp.tile([128, 2, N], F8)
            wm = bigp.tile([128, MT, WWID], F8)

            # Anchors live at cols [GUARD, GUARD+CA) = [256, 1280): the
            # first chunk covers them plus all of h=0's candidates.
            nc.sync.dma_start(ft[:, :, 0:1280], ft_d[:, :, 0:1280])
            nc.sync.dma_start(wm[:], wm_d[:])
            nc.sync.dma_start(ft[:, :, 1280:3072], ft_d[:, :, 1280:3072])
            nc.sync.dma_start(ft[:, :, 3072:4864], ft_d[:, :, 3072:4864])
            nc.sync.dma_start(ft[:, :, 4864:6656], ft_d[:, :, 4864:6656])
            nc.sync.dma_start(ft[:, :, 6656:8192], ft_d[:, :, 6656:8192])

            with ExitStack() as bctx:
                psB = bctx.enter_context(
                    tc.tile_pool(name="psB", bufs=4, space="PSUM"))
                scrp = bctx.enter_context(tc.tile_pool(name="scr", bufs=4))

                wj = [0] * MT  # next window-partial slot per m

                def group(h, m):
                    role = roles[(h, m)]
                    a0 = GUARD + 128 * m
                    lh = ft[:, :, a0:a0 + 128]
                    w0, w1 = _window(m)
                    ps = psB.tile([128, 2, 512], F32, tag="g")
                    for j in range(2):
                        t = 2 * h + j
                        c0, c1 = 512 * t, 512 * (t + 1)
                        lo, hi = max(w0, c0), min(w1, c1)
                        pen = role == "W" and lo < hi
                        nc.tensor.matmul(
                            ps[:, j, :], lh, ft[:, :, c0:c1],
                            start=True, stop=not pen, perf_mode=PM)
                        if pen:
                            nc.tensor.matmul(
                                ps[:, j, lo - c0:hi - c0],
                                negid[:], wm[:, m, lo - w0:hi - w0],
                                start=False, stop=True)
                    if role == "E":
                        nc.scalar.activation(
                            ps[:], ps[:], ACT.Exp, bias=ebias[:],
                            scale=BETA, accum_out=sp2[:, m, h:h + 1])
                        return
                    nc.vector.tensor_reduce(
                        bp2[:, m, h:h + 1], ps[:], axis=AX.XY, op=OP.max)
                    if role == "W":
                        # hardest-pos: min over the penalized window from
                        # f32 PSUM.  The window is contiguous in the
                        # flattened [128, 1024] view even when it straddles
                        # the two 512-col halves.
                        lo0, hi0 = max(w0, 1024 * h), min(w1, 1024 * (h + 1))
                        psf = ps[:].rearrange("p a b -> p (a b)")
                        scr = scrp.tile([128, WWID], F32, tag="scr")
                        j = wj[m]
                        wj[m] += 1
                        nc.vector.tensor_scalar(
                            scr[:, :hi0 - lo0],
                            psf[:, lo0 - 1024 * h:hi0 - 1024 * h],
                            0.0, None, OP.add, OP.min,
                            accum_out=wmn[:, m, j:j + 1])

                for h, m in schedule:
                    group(h, m)

            # ---------------- epilogue ----------------------------------
            with ExitStack() as cctx:
                ep = cctx.enter_context(tc.tile_pool(name="ep", bufs=1))
                psC = cctx.enter_context(
                    tc.tile_pool(name="psC", bufs=1, space="PSUM"))

                negS = ep.tile([128, MT], F32)
                nc.vector.tensor_reduce(
                    negS[:], sp2[:], axis=AX.X, op=OP.add)
                negM = ep.tile([128, MT], F32)
                nc.vector.tensor_reduce(
                    negM[:], bp2[:], axis=AX.X, op=OP.max)
                expM = ep.tile([128, MT], F32)
                nc.scalar.activation(expM[:], negM[:], ACT.Exp,
                                     bias=ebias[:], scale=BETA)
                nc.vector.tensor_add(negS[:], negS[:], expM[:])
                lnS = ep.tile([128, MT], F32)
                nc.scalar.activation(lnS[:], negS[:], ACT.Ln,
                                     bias=zbias[:], scale=1.0)

                wpos = ep.tile([128, MT], F32)
                nc.vector.tensor_reduce(
                    wpos[:], wmn[:], axis=AX.X, op=OP.min)

                # hp2 = clip(2 - 2*(wpos+BIG)); hn2 = clip(2-2*CEN - (2/B)lnS)
                hh2 = ep.tile([128, 2, MT], F32)
                nc.vector.tensor_scalar(
                    hh2[:, 0, :], wpos[:], -2.0, 2.0 - 2.0 * BIG,
                    OP.mult, OP.add)
                nc.vector.tensor_scalar(
                    hh2[:, 1, :], lnS[:], -2.0 / BETA, 2.0 - 2.0 * CEN,
                    OP.mult, OP.add)
                nc.vector.tensor_scalar_max(hh2[:, :, :], hh2[:, :, :],
                                            1e-12)

                # sqrt(x) = exp(0.5 ln x) — measured 1.2e-7 rel on HW,
                # no Newton step needed
                hhf = hh2[:].rearrange("p a b -> p (a b)")
                lnh = ep.tile([128, 2 * MT], F32)
                nc.scalar.activation(lnh[:], hhf, ACT.Ln,
                                     bias=zbias[:], scale=1.0)
                ry = ep.tile([128, 2 * MT], F32)
                nc.scalar.activation(ry[:], lnh[:], ACT.Exp,
                                     bias=zbias[:], scale=0.5)

                ryv = ry[:].rearrange("p (a b) -> p a b", a=2)
                loss = ep.tile([128, MT], F32)
                nc.vector.tensor_sub(loss[:], ryv[:, 0, :], ryv[:, 1, :])
                nc.vector.tensor_scalar(
                    loss[:], loss[:], 0.3, 0.0, OP.add, OP.max)

                rowsum = ep.tile([128, 1], F32)
                nc.vector.tensor_reduce(
                    rowsum[:], loss[:], axis=AX.X, op=OP.add)
                tot = psC.tile([1, 1], F32)
                nc.tensor.matmul(tot[:], rowsum[:], ones[:],
                                 start=True, stop=True)
                osb = ep.tile([1, 1], F32)
                nc.scalar.copy(osb[:], tot[:])
                nc.sync.dma_start(out_d[:], osb[:])

    nc.compile()
    return nc


def _prep_inputs(features, labels):
    feats = np.asarray(features, dtype=np.float32)
    labs = np.asarray(labels)
    order = np.argsort(labs, kind="stable")
    sf = feats[order]
    sl = labs[order]
    nrm = np.maximum(np.sqrt((sf * sf).sum(1, keepdims=True)), 1e-12)
    f8 = (sf / nrm).astype(F8NP)
    s_g = np.searchsorted(sl, sl, side="left")
    e_g = np.searchsorted(sl, sl, side="right")

    # [128, 2, N] with value[p, j, col] = f8[col, 128j + p]
    tmp = np.ascontiguousarray(f8.reshape(N, 2, 128).transpose(2, 1, 0))
    jj = np.arange(WWID)
    in_maps = []
    for c in range(NCORES):
        b = 1024 * c - GUARD
        ft_c = np.roll(tmp, -b, axis=2)
        wm_c = np.zeros((128, MT, WWID), F8NP)
        for m in range(MT):
            i0 = 1024 * c + 128 * m
            ls = s_g[i0:i0 + 128] - b
            le = e_g[i0:i0 + 128] - b
            w0, w1 = _window(m)
            assert (ls >= w0).all() and (le <= w1).all() and (ls < le).all(), \
                f"window containment violated c={c} m={m}"
            wm_c[:, m, :] = (
                (jj[None, :] >= (ls - w0)[:, None])
                & (jj[None, :] < (le - w0)[:, None])).astype(F8NP)
        in_maps.append({"ft": ft_c, "wm": wm_c})
    return in_maps


def kernel(features, labels):
    from concourse.bass_utils import run_bass_kernel_spmd

    if "nc" not in _CACHE:
        _CACHE["nc"] = _build()
    nc = _CACHE["nc"]

    in_maps = _prep_inputs(features, labels)
    res = run_bass_kernel_spmd(nc, in_maps, core_ids=list(range(NCORES)))
    total = np.float64(0.0)
    for c in range(NCORES):
        total += np.float64(res.results[c]["out"].reshape(())[()])
    return np.float32(total / N)
